# revision 15
# baseline (speedup 1.0000x reference)
"""Trainium2 Bass kernel for ClassicalSelfAttention.

  out = softmax((X @ R) @ (X @ E).T / sqrt(D)) @ X,  X: (8192, 1024) fp32

Sharding: sequence-parallel over 8 NeuronCores. Core i owns queries
[i*1024, (i+1)*1024). Each core computes its K^T block (E.T @ X_i^T) in two
512-column halves; each half is AllGathered separately so remote key
sub-blocks become usable ~120us earlier than a single monolithic gather.
Attention runs over 16 key sub-blocks of 512 (ring order, chunk-major),
merged in pairs so the online-softmax merge/rescale cost matches the
8-block version.

QK and the projections run in float32r (~13-bit mantissa at full PE rate);
P is cast to bf16 by the exp activation, halving the LDWEIGHTS cost of the
P transposes and the PV weight loads. Accumulation is fp32 in PSUM.
"""
import numpy as np

import concourse.bass as bass_mod
import concourse.bacc as bacc
import concourse.mybir as mybir
from concourse import tile
from concourse.bass_utils import run_bass_kernel_spmd
from concourse.masks import make_identity

DT = mybir.dt
F32 = DT.float32
F32R = DT.float32r
BF16 = DT.bfloat16
ALU = mybir.AluOpType
ACTF = mybir.ActivationFunctionType

S, D, NCORES = 8192, 1024, 8
SL = S // NCORES          # 1024 queries per core
P = 128                   # partitions
DC = D // P               # 8 contraction chunks
MC = SL // P              # 8 query chunks per core
CB = 512                  # key sub-block size (one collective chunk column)
NV = S // CB              # 16 key sub-blocks
TC = CB // P              # 4 t-chunks per sub-block
SCALE = 1.0 / 32.0        # 1/sqrt(D)
NEG_BIG = -1.0e30


def build_program(n_iter=1, bench=None, num_devices=NCORES):
    nc = bacc.Bacc("TRN2", target_bir_lowering=False, debug=False,
                   num_devices=num_devices)

    xt = nc.declare_dram_parameter("xt", [D, SL], F32R, isOutput=False)
    r_p = nc.declare_dram_parameter("r", [D, D], F32R, isOutput=False)
    e_p = nc.declare_dram_parameter("e", [D, D], F32R, isOutput=False)
    x_p = nc.declare_dram_parameter("x", [S, D], F32R, isOutput=False)
    xbf_p = nc.declare_dram_parameter("xbf", [S, D], BF16, isOutput=False)
    out_p = nc.declare_dram_parameter("out", [SL, D], F32, isOutput=True)

    if bench is None:
        bench = n_iter > 1
    import contextlib
    with tile.TileContext(nc) as tc:
        with (
            tc.tile_pool(name="persist", bufs=1) as pers,
            tc.tile_pool(name="dram", bufs=1, space="DRAM") as dram,
            contextlib.ExitStack() as stack,
        ):
            ktb_own = [dram.tile([D, CB], F32R, name=f"ktb_own{c}")
                       for c in range(2)]
            ktb_all = [dram.tile([NCORES * D, CB], F32R,
                                 addr_space="Local" if bench else "Shared",
                                 name=f"ktb_all{c}")
                       for c in range(2)]
            if bench:
                # touch ktb_all once so in-loop reads see written memory
                for c in range(2):
                    nc.sync.dma_start(
                        ktb_all[c][:],
                        x_p[c * NCORES * D:(c + 1) * NCORES * D, 0:CB]
                        .bitcast(F32R))
            if n_iter > 1:
                stack.enter_context(tc.For_i(0, n_iter, 1))

            qt = pers.tile([P, DC * SL], F32R, tag="qt")       # Q^T, [d|m]
            oacc = pers.tile([P, MC * D], F32, tag="oacc")    # O accum per m
            ident32 = pers.tile([P, P], F32, tag="ident32")
            ident_bf = pers.tile([P, P], BF16, tag="identbf")
            # own K^T stays in SBUF for the b=0 pair (no DRAM round trip)
            kst = [pers.tile([P, DC * CB], F32R, tag=f"kst{h}",
                             name=f"kst{h}") for h in range(2)]
            mst = [[pers.tile([P, 1], F32, tag=f"mst{m}_{j}", name=f"mst{m}_{j}")
                    for j in range(2)] for m in range(MC)]
            sig = [pers.tile([P, 1], F32, tag=f"sig{m}", name=f"sig{m}")
                   for m in range(MC)]

            make_identity(nc, ident32[:])
            nc.vector.tensor_copy(ident_bf[:], ident32[:])
            nc.gpsimd.memset(oacc[:], 0.0)
            for m in range(MC):
                nc.gpsimd.memset(mst[m][0][:], NEG_BIG)
                nc.gpsimd.memset(sig[m][:], 0.0)

            # ---------------- Phase A: projections + chunked allgather ----
            with (
                tc.tile_pool(name="pa", bufs=1) as pa,
                tc.tile_pool(name="pa_ps", bufs=2, space="PSUM") as pa_ps,
            ):
                # xt_sb layout: [p, h * (DC*512)]: h-half of the SL columns,
                # then k-chunk of d_in, then 512 cols
                xt_sb = pa.tile([P, DC * SL], F32R, tag="xt")
                e_sb = pa.tile([P, DC * D], F32R, tag="e")    # [d_in | d_out]
                r_sb = pa.tile([P, DC * D], F32R, tag="r")
                # load order matters: K-proj h0 needs e + xt-h0 first; r last
                nc.sync.dma_start(
                    e_sb.rearrange("p (k c) -> p k c", k=DC),
                    e_p.rearrange("(k p) c -> p k c", p=P))
                for h in range(2):
                    nc.sync.dma_start(
                        xt_sb[:, h * DC * CB:(h + 1) * DC * CB]
                        .rearrange("p (k c) -> p k c", k=DC),
                        xt[:, h * CB:(h + 1) * CB]
                        .rearrange("(k p) c -> p k c", p=P))
                nc.sync.dma_start(
                    r_sb.rearrange("p (k c) -> p k c", k=DC),
                    r_p.rearrange("(k p) c -> p k c", p=P))

                # K^T own block, h-half at a time: kt = E.T @ X_i^T
                for h in range(2):
                    for o in range(DC):
                        ps = pa_ps.tile([P, CB], F32, tag="proj")
                        for k in range(DC):
                            nc.tensor.matmul(
                                ps[:],
                                e_sb[:, k * D + o * P: k * D + (o + 1) * P],
                                xt_sb[:, h * DC * CB + k * CB:
                                      h * DC * CB + (k + 1) * CB],
                                start=(k == 0), stop=(k == DC - 1),
                            )
                        nc.vector.tensor_copy(
                            kst[h][:, o * CB:(o + 1) * CB], ps[:])
                    nc.sync.dma_start(
                        ktb_own[h].rearrange("(o p) c -> p o c", p=P),
                        kst[h].rearrange("p (o c) -> p o c", o=DC))
                    if bench:
                        # stand-in for the collective with similar traffic
                        nc.gpsimd.dma_start(ktb_all[h][0:D, :], ktb_own[h][:])
                    else:
                        nc.gpsimd.collective_compute(
                            "AllGather",
                            ALU.bypass,
                            replica_groups=[list(range(NCORES))],
                            ins=[ktb_own[h].opt()],
                            outs=[ktb_all[h].opt()],
                        )

                # Q^T: qt = R.T @ X_i^T   [d_out, m]
                for o in range(DC):
                    for h in range(2):
                        ps = pa_ps.tile([P, CB], F32, tag="proj")
                        for k in range(DC):
                            nc.tensor.matmul(
                                ps[:],
                                r_sb[:, k * D + o * P: k * D + (o + 1) * P],
                                xt_sb[:, h * DC * CB + k * CB:
                                      h * DC * CB + (k + 1) * CB],
                                start=(k == 0), stop=(k == DC - 1),
                            )
                        nc.vector.tensor_copy(
                            qt[:, o * SL + h * CB: o * SL + (h + 1) * CB],
                            ps[:])

            # ---------------- Phase B: blocked attention -----------------
            # 16 key sub-blocks of 512 (chunk-major ring order), merged in
            # pairs for the online-softmax update. Software-pipelined: PE
            # runs transposes+PV of a previous pair while DVE/ACT compute
            # stats+exp of the current one.
            with (
                tc.tile_pool(name="kt", bufs=3) as ktp,
                tc.tile_pool(name="xb", bufs=4) as xbp,
                tc.tile_pool(name="ph", bufs=4) as php,
                tc.tile_pool(name="pt", bufs=2) as ptp,
                tc.tile_pool(name="of", bufs=2) as ofp,
                tc.tile_pool(name="stats", bufs=6) as stp,
                tc.tile_pool(name="s_ps", bufs=4, space="PSUM") as sps,
                tc.tile_pool(name="t_ps", bufs=2, space="PSUM") as tps,
                tc.tile_pool(name="o_ps", bufs=1, space="PSUM") as ops,
            ):
                def flush_pe(pend):
                    phs, alpha, m, v, xbs = pend
                    o_part = ops.tile([P, D], F32, tag="opart", name="o_part")
                    pts = []
                    for s in range(2):
                        tp = tps.tile([P, CB], BF16, tag="tp", name="tp")
                        for cc in range(TC):
                            nc.tensor.transpose(
                                tp[:, cc * P:(cc + 1) * P],
                                phs[s][:, cc * P:(cc + 1) * P],
                                ident_bf[:],
                            )
                        pt = ptp.tile([P, CB], BF16, tag="pt", name="pt")
                        nc.scalar.copy(pt[:], tp[:])
                        pts.append(pt)
                    for s in range(2):
                        for cc in range(TC):
                            for h in range(D // CB):
                                nc.tensor.matmul(
                                    o_part[:, h * CB:(h + 1) * CB],
                                    pts[s][:, cc * P:(cc + 1) * P],
                                    xbs[s][:, cc * D + h * CB:
                                           cc * D + (h + 1) * CB],
                                    start=(s == 0 and cc == 0),
                                    stop=(s == 1 and cc == TC - 1),
                                )
                    return o_part

                def flush_dve(pend, o_part):
                    phs, alpha, m, v, xbs = pend
                    nc.vector.scalar_tensor_tensor(
                        oacc[:, m * D:(m + 1) * D],
                        oacc[:, m * D:(m + 1) * D],
                        alpha[:], o_part[:],
                        op0=ALU.mult, op1=ALU.add)
                    if v == NV // 2 - 1:
                        # finalize this m: divide by softmax sum and store
                        rcp = stp.tile([P, 1], F32, tag="rcp", name="rcp")
                        nc.vector.reciprocal(rcp[:], sig[m][:])
                        of = ofp.tile([P, D], F32, tag="ofin", name="ofin")
                        nc.vector.tensor_scalar_mul(
                            of[:], oacc[:, m * D:(m + 1) * D], rcp[:])
                        nc.sync.dma_start(out_p[m * P:(m + 1) * P, :], of[:])

                pending = []
                pid = nc.sync.partition_id()
                # sub-block visit order: own block's two halves first (no
                # collective dependency), then chunk-major ring order so
                # chunk-0 sub-blocks are consumed while chunk 1 gathers.
                visits = [(0, 0), (0, 1)]
                for c in range(2):
                    visits += [(b, c) for b in range(1, NCORES)]
                for v in range(NV // 2):
                    sub = [visits[2 * v], visits[2 * v + 1]]
                    kts, xbs = [], []
                    for (b, c) in sub:
                        if b == 0:
                            # own K^T already sits in SBUF from phase A
                            kts.append(kst[c])
                        else:
                            kt = ktp.tile([P, DC * CB], F32R, tag="kt",
                                          name="kt")
                            nc.sync.dma_start(
                                kt.rearrange("p (k c) -> p k c", k=DC),
                                ktb_all[c][
                                    bass_mod.ds(((pid + b) % NCORES) * D, D), :]
                                .rearrange("(k p) c -> p k c", p=P))
                            kts.append(kt)
                        # PV runs in bf16 (matmul can't mix f32r/bf16); X is
                        # pre-cast to bf16 on the host
                        xb = xbp.tile([P, TC * D], BF16, tag="xb", name="xb")
                        nc.sync.dma_start(
                            xb.rearrange("p (k c) -> p k c", k=TC),
                            xbf_p[bass_mod.ds(
                                ((pid + b) % NCORES) * SL + c * CB, CB), :]
                            .rearrange("(k p) c -> p k c", p=P))
                        xbs.append(xb)

                    for m in range(MC):
                        sh_ = [sps.tile([P, CB], F32, tag="s", name="s")
                               for _ in range(2)]
                        mqh = [stp.tile([P, 1], F32, tag=f"mq{s}",
                                        name=f"mq{s}") for s in range(2)]
                        for s in range(2):
                            for k in range(DC):
                                nc.tensor.matmul(
                                    sh_[s][:],
                                    qt[:, k * SL + m * P: k * SL + (m + 1) * P],
                                    kts[s][:, k * CB:(k + 1) * CB],
                                    start=(k == 0), stop=(k == DC - 1),
                                )
                            nc.vector.reduce_max(mqh[s][:], sh_[s][:],
                                                 axis=mybir.AxisListType.X)

                        # online softmax stats; mst ping-pongs on v parity
                        m_old = mst[m][v % 2]
                        mnew = mst[m][(v + 1) % 2]
                        mq = stp.tile([P, 1], F32, tag="mq", name="mq")
                        nc.vector.tensor_max(mq[:], mqh[0][:], mqh[1][:])
                        nc.vector.tensor_max(mnew[:], m_old[:], mq[:])
                        nbias = stp.tile([P, 1], F32, tag="nbias", name="nbias")
                        nc.scalar.mul(nbias[:], mnew[:], -SCALE)
                        # alpha = exp((m_old - mnew)/32)
                        alpha = stp.tile([P, 1], F32, tag="alpha", name="alpha")
                        nc.scalar.activation(alpha[:], m_old[:], ACTF.Exp,
                                             bias=nbias[:], scale=SCALE)

                        # phat = exp(s/32 - mnew/32) in bf16; sums into sq
                        phs = []
                        sqh = [stp.tile([P, 1], F32, tag=f"sq{s}",
                                        name=f"sq{s}") for s in range(2)]
                        for s in range(2):
                            ph = php.tile([P, CB], BF16, tag="ph", name="ph")
                            nc.scalar.activation(ph[:], sh_[s][:], ACTF.Exp,
                                                 bias=nbias[:], scale=SCALE,
                                                 accum_out=sqh[s][:])
                            phs.append(ph)
                        sq = stp.tile([P, 1], F32, tag="sq", name="sq")
                        nc.vector.tensor_add(sq[:], sqh[0][:], sqh[1][:])
                        nc.vector.scalar_tensor_tensor(
                            sig[m][:], sig[m][:], alpha[:], sq[:],
                            op0=ALU.mult, op1=ALU.add)

                        pending.append((phs, alpha, m, v, xbs))
                        if len(pending) > 2:
                            pend_fl = pending.pop(0)
                            flush_dve(pend_fl, flush_pe(pend_fl))
                for pend in pending:
                    flush_dve(pend, flush_pe(pend))

    nc.compile()
    return nc


_PROGRAM = None


def _get_program():
    global _PROGRAM
    if _PROGRAM is None:
        _PROGRAM = build_program()
    return _PROGRAM


def kernel(inputs, rotation_params, entangle_params, _trace=False):
    X = np.ascontiguousarray(np.asarray(inputs, dtype=np.float32))
    R = np.ascontiguousarray(np.asarray(rotation_params, dtype=np.float32))
    E = np.ascontiguousarray(np.asarray(entangle_params, dtype=np.float32))
    assert X.shape == (S, D) and R.shape == (D, D) and E.shape == (D, D)

    import ml_dtypes
    XT = np.ascontiguousarray(X.T)
    Xbf = np.ascontiguousarray(X.astype(ml_dtypes.bfloat16))
    in_maps = []
    for i in range(NCORES):
        in_maps.append({
            "xt": np.ascontiguousarray(XT[:, i * SL:(i + 1) * SL]),
            "r": R,
            "e": E,
            "x": X,
            "xbf": Xbf,
        })

    nc = _get_program()
    res = run_bass_kernel_spmd(nc, in_maps, list(range(NCORES)),
                               trace=_trace)
    out = np.concatenate([res.results[i]["out"] for i in range(NCORES)],
                         axis=0)
    if _trace:
        return out, res
    return out


# revision 17
# speedup vs baseline: 1.0931x; 1.0931x over previous
"""Trainium2 Bass kernel for ClassicalSelfAttention.

  out = softmax((X @ R) @ (X @ E).T / sqrt(D)) @ X,  X: (8192, 1024) fp32

Sharding: sequence-parallel over 8 NeuronCores. Core i owns queries
[i*1024, (i+1)*1024). Each core computes its K^T block (E.T @ X_i^T) in two
512-column halves; each half is AllGathered separately so remote key
sub-blocks become usable ~120us earlier than a single monolithic gather.
Attention runs over 16 key sub-blocks of 512 (ring order, chunk-major),
merged in pairs so the online-softmax merge/rescale cost matches the
8-block version.

QK and the projections run in float32r (~13-bit mantissa at full PE rate);
P is cast to bf16 by the exp activation, halving the LDWEIGHTS cost of the
P transposes and the PV weight loads. Accumulation is fp32 in PSUM.
"""
import numpy as np

import concourse.bass as bass_mod
import concourse.bacc as bacc
import concourse.mybir as mybir
from concourse import tile
from concourse.bass_utils import run_bass_kernel_spmd
from concourse.masks import make_identity

DT = mybir.dt
F32 = DT.float32
F32R = DT.float32r
BF16 = DT.bfloat16
ALU = mybir.AluOpType
ACTF = mybir.ActivationFunctionType

S, D, NCORES = 8192, 1024, 8
SL = S // NCORES          # 1024 queries per core
P = 128                   # partitions
DC = D // P               # 8 contraction chunks
MC = SL // P              # 8 query chunks per core
CB = 512                  # key sub-block size (one collective chunk column)
NV = S // CB              # 16 key sub-blocks
TC = CB // P              # 4 t-chunks per sub-block
SCALE = 1.0 / 32.0        # 1/sqrt(D)
NEG_BIG = -1.0e30


def build_program(n_iter=1, bench=None, num_devices=NCORES):
    nc = bacc.Bacc("TRN2", target_bir_lowering=False, debug=False,
                   num_devices=num_devices)

    xt = nc.declare_dram_parameter("xt", [D, SL], F32R, isOutput=False)
    r_p = nc.declare_dram_parameter("r", [D, D], F32R, isOutput=False)
    e_p = nc.declare_dram_parameter("e", [D, D], F32R, isOutput=False)
    x_p = nc.declare_dram_parameter("x", [S, D], F32R, isOutput=False)
    xbf_p = nc.declare_dram_parameter("xbf", [S, D], BF16, isOutput=False)
    out_p = nc.declare_dram_parameter("out", [SL, D], F32, isOutput=True)

    if bench is None:
        bench = n_iter > 1
    import contextlib
    with tile.TileContext(nc) as tc:
        with (
            tc.tile_pool(name="persist", bufs=1) as pers,
            tc.tile_pool(name="dram", bufs=1, space="DRAM") as dram,
            contextlib.ExitStack() as stack,
        ):
            ktb_own = [dram.tile([D, CB], F32R, name=f"ktb_own{c}")
                       for c in range(2)]
            ktb_all = [dram.tile([NCORES * D, CB], F32R,
                                 addr_space="Local" if bench else "Shared",
                                 name=f"ktb_all{c}")
                       for c in range(2)]
            if bench:
                # touch ktb_all once so in-loop reads see written memory
                for c in range(2):
                    nc.sync.dma_start(
                        ktb_all[c][:],
                        x_p[c * NCORES * D:(c + 1) * NCORES * D, 0:CB]
                        .bitcast(F32R))
            if n_iter > 1:
                stack.enter_context(tc.For_i(0, n_iter, 1))

            qt = pers.tile([P, DC * SL], F32R, tag="qt")       # Q^T, [d|m]
            oacc = pers.tile([P, MC * D], F32, tag="oacc")    # O accum per m
            ident32 = pers.tile([P, P], F32, tag="ident32")
            ident_bf = pers.tile([P, P], BF16, tag="identbf")
            # own K^T stays in SBUF for the b=0 pair (no DRAM round trip)
            kst = [pers.tile([P, DC * CB], F32R, tag=f"kst{h}",
                             name=f"kst{h}") for h in range(2)]
            mst = [[pers.tile([P, 1], F32, tag=f"mst{m}_{j}", name=f"mst{m}_{j}")
                    for j in range(2)] for m in range(MC)]
            sig = [pers.tile([P, 1], F32, tag=f"sig{m}", name=f"sig{m}")
                   for m in range(MC)]

            make_identity(nc, ident32[:])
            nc.vector.tensor_copy(ident_bf[:], ident32[:])
            nc.gpsimd.memset(oacc[:], 0.0)
            for m in range(MC):
                nc.gpsimd.memset(mst[m][0][:], NEG_BIG)
                nc.gpsimd.memset(sig[m][:], 0.0)

            # ---------------- Phase A: projections + chunked allgather ----
            with (
                tc.tile_pool(name="pa", bufs=1) as pa,
                tc.tile_pool(name="pa_ps", bufs=2, space="PSUM") as pa_ps,
            ):
                # xt_sb layout: [p, h * (DC*512)]: h-half of the SL columns,
                # then k-chunk of d_in, then 512 cols
                xt_sb = pa.tile([P, DC * SL], F32R, tag="xt")
                e_sb = pa.tile([P, DC * D], F32R, tag="e")    # [d_in | d_out]
                r_sb = pa.tile([P, DC * D], F32R, tag="r")
                # load order matters: K-proj h0 needs e + xt-h0 first; r last
                nc.sync.dma_start(
                    e_sb.rearrange("p (k c) -> p k c", k=DC),
                    e_p.rearrange("(k p) c -> p k c", p=P))
                for h in range(2):
                    nc.sync.dma_start(
                        xt_sb[:, h * DC * CB:(h + 1) * DC * CB]
                        .rearrange("p (k c) -> p k c", k=DC),
                        xt[:, h * CB:(h + 1) * CB]
                        .rearrange("(k p) c -> p k c", p=P))
                nc.sync.dma_start(
                    r_sb.rearrange("p (k c) -> p k c", k=DC),
                    r_p.rearrange("(k p) c -> p k c", p=P))

                # K^T own block, h-half at a time: kt = E.T @ X_i^T
                for h in range(2):
                    for o in range(DC):
                        ps = pa_ps.tile([P, CB], F32, tag="proj")
                        for k in range(DC):
                            nc.tensor.matmul(
                                ps[:],
                                e_sb[:, k * D + o * P: k * D + (o + 1) * P],
                                xt_sb[:, h * DC * CB + k * CB:
                                      h * DC * CB + (k + 1) * CB],
                                start=(k == 0), stop=(k == DC - 1),
                            )
                        nc.vector.tensor_copy(
                            kst[h][:, o * CB:(o + 1) * CB], ps[:])
                    nc.sync.dma_start(
                        ktb_own[h].rearrange("(o p) c -> p o c", p=P),
                        kst[h].rearrange("p (o c) -> p o c", o=DC))
                    if bench:
                        # stand-in for the collective with similar traffic
                        nc.gpsimd.dma_start(ktb_all[h][0:D, :], ktb_own[h][:])
                    else:
                        nc.gpsimd.collective_compute(
                            "AllGather",
                            ALU.bypass,
                            replica_groups=[list(range(NCORES))],
                            ins=[ktb_own[h].opt()],
                            outs=[ktb_all[h].opt()],
                        )

                # Q^T: qt = R.T @ X_i^T   [d_out, m]
                for o in range(DC):
                    for h in range(2):
                        ps = pa_ps.tile([P, CB], F32, tag="proj")
                        for k in range(DC):
                            nc.tensor.matmul(
                                ps[:],
                                r_sb[:, k * D + o * P: k * D + (o + 1) * P],
                                xt_sb[:, h * DC * CB + k * CB:
                                      h * DC * CB + (k + 1) * CB],
                                start=(k == 0), stop=(k == DC - 1),
                            )
                        nc.vector.tensor_copy(
                            qt[:, o * SL + h * CB: o * SL + (h + 1) * CB],
                            ps[:])

            # ---------------- Phase B: blocked attention -----------------
            # 16 key sub-blocks of 512 (chunk-major ring order), merged in
            # pairs for the online-softmax update. Software-pipelined: PE
            # runs transposes+PV of a previous pair while DVE/ACT compute
            # stats+exp of the current one.
            with (
                tc.tile_pool(name="kt", bufs=4) as ktp,
                tc.tile_pool(name="xb", bufs=3) as xbp,
                tc.tile_pool(name="ph", bufs=4) as php,
                tc.tile_pool(name="pt", bufs=2) as ptp,
                tc.tile_pool(name="of", bufs=2) as ofp,
                tc.tile_pool(name="stats", bufs=6) as stp,
                tc.tile_pool(name="s_ps", bufs=4, space="PSUM") as sps,
                tc.tile_pool(name="t_ps", bufs=2, space="PSUM") as tps,
                tc.tile_pool(name="o_ps", bufs=1, space="PSUM") as ops,
            ):
                def flush_pe(pend):
                    phs, alpha, m, v, xbs = pend
                    o_part = ops.tile([P, D], F32, tag="opart", name="o_part")
                    pts = []
                    for s in range(2):
                        tp = tps.tile([P, CB], BF16, tag="tp", name="tp")
                        for cc in range(TC):
                            nc.tensor.transpose(
                                tp[:, cc * P:(cc + 1) * P],
                                phs[s][:, cc * P:(cc + 1) * P],
                                ident_bf[:],
                            )
                        pt = ptp.tile([P, CB], BF16, tag="pt", name="pt")
                        nc.scalar.copy(pt[:], tp[:])
                        pts.append(pt)
                    for s in range(2):
                        for cc in range(TC):
                            for h in range(D // CB):
                                nc.tensor.matmul(
                                    o_part[:, h * CB:(h + 1) * CB],
                                    pts[s][:, cc * P:(cc + 1) * P],
                                    xbs[s][:, cc * D + h * CB:
                                           cc * D + (h + 1) * CB],
                                    start=(s == 0 and cc == 0),
                                    stop=(s == 1 and cc == TC - 1),
                                )
                    return o_part

                def flush_dve(pend, o_part):
                    phs, alpha, m, v, xbs = pend
                    nc.vector.scalar_tensor_tensor(
                        oacc[:, m * D:(m + 1) * D],
                        oacc[:, m * D:(m + 1) * D],
                        alpha[:], o_part[:],
                        op0=ALU.mult, op1=ALU.add)
                    if v == NV // 2 - 1:
                        # finalize this m: divide by softmax sum and store
                        rcp = stp.tile([P, 1], F32, tag="rcp", name="rcp")
                        nc.vector.reciprocal(rcp[:], sig[m][:])
                        of = ofp.tile([P, D], F32, tag="ofin", name="ofin")
                        nc.vector.tensor_scalar_mul(
                            of[:], oacc[:, m * D:(m + 1) * D], rcp[:])
                        nc.sync.dma_start(out_p[m * P:(m + 1) * P, :], of[:])

                pending = []
                pid = nc.sync.partition_id()
                # sub-block visit order: own block's two halves first (no
                # collective dependency), then chunk-major ring order so
                # chunk-0 sub-blocks are consumed while chunk 1 gathers.
                visits = [(0, 0), (0, 1)]
                for c in range(2):
                    visits += [(b, c) for b in range(1, NCORES)]
                for v in range(NV // 2):
                    sub = [visits[2 * v], visits[2 * v + 1]]
                    kts, xbs = [], []
                    for (b, c) in sub:
                        if b == 0:
                            # own K^T already sits in SBUF from phase A
                            kts.append(kst[c])
                        else:
                            kt = ktp.tile([P, DC * CB], F32R, tag="kt",
                                          name="kt")
                            nc.sync.dma_start(
                                kt.rearrange("p (k c) -> p k c", k=DC),
                                ktb_all[c][
                                    bass_mod.ds(((pid + b) % NCORES) * D, D), :]
                                .rearrange("(k p) c -> p k c", p=P))
                            kts.append(kt)
                        # PV runs in bf16 (matmul can't mix f32r/bf16); X is
                        # pre-cast to bf16 on the host
                        xb = xbp.tile([P, TC * D], BF16, tag="xb", name="xb")
                        nc.sync.dma_start(
                            xb.rearrange("p (k c) -> p k c", k=TC),
                            xbf_p[bass_mod.ds(
                                ((pid + b) % NCORES) * SL + c * CB, CB), :]
                            .rearrange("(k p) c -> p k c", p=P))
                        xbs.append(xb)

                    for m in range(MC):
                        sh_ = [sps.tile([P, CB], F32, tag="s", name="s")
                               for _ in range(2)]
                        mqh = [stp.tile([P, 1], F32, tag=f"mq{s}",
                                        name=f"mq{s}") for s in range(2)]
                        for s in range(2):
                            for k in range(DC):
                                nc.tensor.matmul(
                                    sh_[s][:],
                                    qt[:, k * SL + m * P: k * SL + (m + 1) * P],
                                    kts[s][:, k * CB:(k + 1) * CB],
                                    start=(k == 0), stop=(k == DC - 1),
                                )
                            nc.vector.reduce_max(mqh[s][:], sh_[s][:],
                                                 axis=mybir.AxisListType.X)

                        # online softmax stats; mst ping-pongs on v parity
                        m_old = mst[m][v % 2]
                        mnew = mst[m][(v + 1) % 2]
                        mq = stp.tile([P, 1], F32, tag="mq", name="mq")
                        nc.vector.tensor_max(mq[:], mqh[0][:], mqh[1][:])
                        nc.vector.tensor_max(mnew[:], m_old[:], mq[:])
                        nbias = stp.tile([P, 1], F32, tag="nbias", name="nbias")
                        nc.scalar.mul(nbias[:], mnew[:], -SCALE)
                        # alpha = exp((m_old - mnew)/32)
                        alpha = stp.tile([P, 1], F32, tag="alpha", name="alpha")
                        nc.scalar.activation(alpha[:], m_old[:], ACTF.Exp,
                                             bias=nbias[:], scale=SCALE)

                        # phat = exp(s/32 - mnew/32) in bf16; sums into sq
                        phs = []
                        sqh = [stp.tile([P, 1], F32, tag=f"sq{s}",
                                        name=f"sq{s}") for s in range(2)]
                        for s in range(2):
                            ph = php.tile([P, CB], BF16, tag="ph", name="ph")
                            nc.scalar.activation(ph[:], sh_[s][:], ACTF.Exp,
                                                 bias=nbias[:], scale=SCALE,
                                                 accum_out=sqh[s][:])
                            phs.append(ph)
                        sq = stp.tile([P, 1], F32, tag="sq", name="sq")
                        nc.vector.tensor_add(sq[:], sqh[0][:], sqh[1][:])
                        nc.vector.scalar_tensor_tensor(
                            sig[m][:], sig[m][:], alpha[:], sq[:],
                            op0=ALU.mult, op1=ALU.add)

                        pending.append((phs, alpha, m, v, xbs))
                        if len(pending) > 2:
                            pend_fl = pending.pop(0)
                            flush_dve(pend_fl, flush_pe(pend_fl))
                for pend in pending:
                    flush_dve(pend, flush_pe(pend))

    nc.compile()
    return nc


_PROGRAM = None


def _get_program():
    global _PROGRAM
    if _PROGRAM is None:
        _PROGRAM = build_program()
    return _PROGRAM


def kernel(inputs, rotation_params, entangle_params, _trace=False):
    X = np.ascontiguousarray(np.asarray(inputs, dtype=np.float32))
    R = np.ascontiguousarray(np.asarray(rotation_params, dtype=np.float32))
    E = np.ascontiguousarray(np.asarray(entangle_params, dtype=np.float32))
    assert X.shape == (S, D) and R.shape == (D, D) and E.shape == (D, D)

    import ml_dtypes
    XT = np.ascontiguousarray(X.T)
    Xbf = np.ascontiguousarray(X.astype(ml_dtypes.bfloat16))
    in_maps = []
    for i in range(NCORES):
        in_maps.append({
            "xt": np.ascontiguousarray(XT[:, i * SL:(i + 1) * SL]),
            "r": R,
            "e": E,
            "x": X,
            "xbf": Xbf,
        })

    nc = _get_program()
    res = run_bass_kernel_spmd(nc, in_maps, list(range(NCORES)),
                               trace=_trace)
    out = np.concatenate([res.results[i]["out"] for i in range(NCORES)],
                         axis=0)
    if _trace:
        return out, res
    return out


# revision 23
# speedup vs baseline: 1.0939x; 1.0008x over previous
"""Trainium2 Bass kernel for ClassicalSelfAttention.

  out = softmax((X @ R) @ (X @ E).T / sqrt(D)) @ X,  X: (8192, 1024) fp32

Sharding: sequence-parallel over 8 NeuronCores. Core i owns queries
[i*1024, (i+1)*1024). Each core computes its K^T block (E.T @ X_i^T) in two
512-column halves; each half is AllGathered separately so remote key
sub-blocks become usable ~120us earlier than a single monolithic gather.
Attention runs over 16 key sub-blocks of 512 (ring order, chunk-major),
merged in pairs so the online-softmax merge/rescale cost matches the
8-block version.

QK and the projections run in float32r (~13-bit mantissa at full PE rate);
P is cast to bf16 by the exp activation, halving the LDWEIGHTS cost of the
P transposes and the PV weight loads. Accumulation is fp32 in PSUM.
"""
import numpy as np

import concourse.bass as bass_mod
import concourse.bacc as bacc
import concourse.mybir as mybir
from concourse import tile
from concourse.bass_utils import run_bass_kernel_spmd
from concourse.masks import make_identity

DT = mybir.dt
F32 = DT.float32
F32R = DT.float32r
BF16 = DT.bfloat16
ALU = mybir.AluOpType
ACTF = mybir.ActivationFunctionType

S, D, NCORES = 8192, 1024, 8
SL = S // NCORES          # 1024 queries per core
P = 128                   # partitions
DC = D // P               # 8 contraction chunks
MC = SL // P              # 8 query chunks per core
CB = 512                  # key sub-block size (one collective chunk column)
NV = S // CB              # 16 key sub-blocks
TC = CB // P              # 4 t-chunks per sub-block
SCALE = 1.0 / 32.0        # 1/sqrt(D)
NEG_BIG = -1.0e30


def build_program(n_iter=1, bench=None, num_devices=NCORES):
    nc = bacc.Bacc("TRN2", target_bir_lowering=False, debug=False,
                   num_devices=num_devices)

    xt = nc.declare_dram_parameter("xt", [D, SL], F32R, isOutput=False)
    r_p = nc.declare_dram_parameter("r", [D, D], F32R, isOutput=False)
    e_p = nc.declare_dram_parameter("e", [D, D], F32R, isOutput=False)
    x_p = nc.declare_dram_parameter("x", [S, D], F32R, isOutput=False)
    xbf_p = nc.declare_dram_parameter("xbf", [S, D], BF16, isOutput=False)
    out_p = nc.declare_dram_parameter("out", [SL, D], F32, isOutput=True)

    if bench is None:
        bench = n_iter > 1
    import contextlib
    with tile.TileContext(nc) as tc:
        with (
            tc.tile_pool(name="persist", bufs=1) as pers,
            tc.tile_pool(name="dram", bufs=1, space="DRAM") as dram,
            contextlib.ExitStack() as stack,
        ):
            ktb_own = [dram.tile([D, CB], F32R, name=f"ktb_own{c}")
                       for c in range(2)]
            ktb_all = [dram.tile([NCORES * D, CB], F32R,
                                 addr_space="Local" if bench else "Shared",
                                 name=f"ktb_all{c}")
                       for c in range(2)]
            if bench:
                # touch ktb_all once so in-loop reads see written memory
                for c in range(2):
                    nc.sync.dma_start(
                        ktb_all[c][:],
                        x_p[c * NCORES * D:(c + 1) * NCORES * D, 0:CB]
                        .bitcast(F32R))
            if n_iter > 1:
                stack.enter_context(tc.For_i(0, n_iter, 1))

            qt = pers.tile([P, DC * SL], F32R, tag="qt")       # Q^T, [d|m]
            oacc = pers.tile([P, MC * D], F32, tag="oacc")    # O accum per m
            ident32 = pers.tile([P, P], F32, tag="ident32")
            ident_bf = pers.tile([P, P], BF16, tag="identbf")
            # own K^T stays in SBUF for the b=0 pair (no DRAM round trip)
            kst = [pers.tile([P, DC * CB], F32R, tag=f"kst{h}",
                             name=f"kst{h}") for h in range(2)]
            mst = [[pers.tile([P, 1], F32, tag=f"mst{m}_{j}", name=f"mst{m}_{j}")
                    for j in range(2)] for m in range(MC)]
            sig = [pers.tile([P, 1], F32, tag=f"sig{m}", name=f"sig{m}")
                   for m in range(MC)]

            make_identity(nc, ident32[:])
            nc.vector.tensor_copy(ident_bf[:], ident32[:])
            nc.gpsimd.memset(oacc[:], 0.0)
            for m in range(MC):
                nc.gpsimd.memset(mst[m][0][:], NEG_BIG)
                nc.gpsimd.memset(sig[m][:], 0.0)

            # ---------------- Phase A: projections + chunked allgather ----
            with (
                tc.tile_pool(name="pa", bufs=1) as pa,
                tc.tile_pool(name="pa_ps", bufs=2, space="PSUM") as pa_ps,
            ):
                # xt_sb layout: [p, h * (DC*512)]: h-half of the SL columns,
                # then k-chunk of d_in, then 512 cols
                xt_sb = pa.tile([P, DC * SL], F32R, tag="xt")
                e_sb = pa.tile([P, DC * D], F32R, tag="e")    # [d_in | d_out]
                r_sb = pa.tile([P, DC * D], F32R, tag="r")
                # parallel DMA queues (one per trigger engine) so K-proj h0's
                # deps (e + xt-h0) land as fast as possible
                nc.sync.dma_start(
                    e_sb.rearrange("p (k c) -> p k c", k=DC),
                    e_p.rearrange("(k p) c -> p k c", p=P))
                for h, eng in ((0, nc.scalar), (1, nc.gpsimd)):
                    eng.dma_start(
                        xt_sb[:, h * DC * CB:(h + 1) * DC * CB]
                        .rearrange("p (k c) -> p k c", k=DC),
                        xt[:, h * CB:(h + 1) * CB]
                        .rearrange("(k p) c -> p k c", p=P))
                nc.scalar.dma_start(
                    r_sb.rearrange("p (k c) -> p k c", k=DC),
                    r_p.rearrange("(k p) c -> p k c", p=P))

                # K^T own block, h-half at a time: kt = E.T @ X_i^T
                for h in range(2):
                    for o in range(DC):
                        ps = pa_ps.tile([P, CB], F32, tag="proj")
                        for k in range(DC):
                            nc.tensor.matmul(
                                ps[:],
                                e_sb[:, k * D + o * P: k * D + (o + 1) * P],
                                xt_sb[:, h * DC * CB + k * CB:
                                      h * DC * CB + (k + 1) * CB],
                                start=(k == 0), stop=(k == DC - 1),
                            )
                        nc.vector.tensor_copy(
                            kst[h][:, o * CB:(o + 1) * CB], ps[:])
                    nc.sync.dma_start(
                        ktb_own[h].rearrange("(o p) c -> p o c", p=P),
                        kst[h].rearrange("p (o c) -> p o c", o=DC))
                    if bench:
                        # stand-in for the collective with similar traffic
                        nc.gpsimd.dma_start(ktb_all[h][0:D, :], ktb_own[h][:])
                    else:
                        nc.gpsimd.collective_compute(
                            "AllGather",
                            ALU.bypass,
                            replica_groups=[list(range(NCORES))],
                            ins=[ktb_own[h].opt()],
                            outs=[ktb_all[h].opt()],
                        )

                # Q^T: qt = R.T @ X_i^T   [d_out, m]
                for o in range(DC):
                    for h in range(2):
                        ps = pa_ps.tile([P, CB], F32, tag="proj")
                        for k in range(DC):
                            nc.tensor.matmul(
                                ps[:],
                                r_sb[:, k * D + o * P: k * D + (o + 1) * P],
                                xt_sb[:, h * DC * CB + k * CB:
                                      h * DC * CB + (k + 1) * CB],
                                start=(k == 0), stop=(k == DC - 1),
                            )
                        nc.vector.tensor_copy(
                            qt[:, o * SL + h * CB: o * SL + (h + 1) * CB],
                            ps[:])

            # ---------------- Phase B: blocked attention -----------------
            # 16 key sub-blocks of 512 (chunk-major ring order), merged in
            # pairs for the online-softmax update. Software-pipelined: PE
            # runs transposes+PV of a previous pair while DVE/ACT compute
            # stats+exp of the current one.
            with (
                tc.tile_pool(name="kt", bufs=4) as ktp,
                tc.tile_pool(name="xb", bufs=3) as xbp,
                tc.tile_pool(name="ph", bufs=4) as php,
                tc.tile_pool(name="pt", bufs=2) as ptp,
                tc.tile_pool(name="of", bufs=2) as ofp,
                tc.tile_pool(name="stats", bufs=6) as stp,
                tc.tile_pool(name="s_ps", bufs=4, space="PSUM") as sps,
                tc.tile_pool(name="t_ps", bufs=2, space="PSUM") as tps,
                tc.tile_pool(name="o_ps", bufs=1, space="PSUM") as ops,
            ):
                def flush_pe(pend):
                    phs, alpha, m, v, xbs = pend
                    o_part = ops.tile([P, D], F32, tag="opart", name="o_part")
                    pts = []
                    for s in range(2):
                        tp = tps.tile([P, CB], BF16, tag="tp", name="tp")
                        for cc in range(TC):
                            nc.tensor.transpose(
                                tp[:, cc * P:(cc + 1) * P],
                                phs[s][:, cc * P:(cc + 1) * P],
                                ident_bf[:],
                            )
                        pt = ptp.tile([P, CB], BF16, tag="pt", name="pt")
                        nc.scalar.copy(pt[:], tp[:])
                        pts.append(pt)
                    for s in range(2):
                        for cc in range(TC):
                            for h in range(D // CB):
                                nc.tensor.matmul(
                                    o_part[:, h * CB:(h + 1) * CB],
                                    pts[s][:, cc * P:(cc + 1) * P],
                                    xbs[s][:, cc * D + h * CB:
                                           cc * D + (h + 1) * CB],
                                    start=(s == 0 and cc == 0),
                                    stop=(s == 1 and cc == TC - 1),
                                )
                    return o_part

                def flush_dve(pend, o_part):
                    phs, alpha, m, v, xbs = pend
                    nc.vector.scalar_tensor_tensor(
                        oacc[:, m * D:(m + 1) * D],
                        oacc[:, m * D:(m + 1) * D],
                        alpha[:], o_part[:],
                        op0=ALU.mult, op1=ALU.add)
                    if v == NV // 2 - 1:
                        # finalize this m: divide by softmax sum and store
                        rcp = stp.tile([P, 1], F32, tag="rcp", name="rcp")
                        nc.vector.reciprocal(rcp[:], sig[m][:])
                        of = ofp.tile([P, D], F32, tag="ofin", name="ofin")
                        nc.vector.tensor_scalar_mul(
                            of[:], oacc[:, m * D:(m + 1) * D], rcp[:])
                        nc.sync.dma_start(out_p[m * P:(m + 1) * P, :], of[:])

                pending = []
                pid = nc.sync.partition_id()
                pid_a = nc.scalar.partition_id()
                pid_g = nc.gpsimd.partition_id()
                # sub-block visit order: own block's two halves first (no
                # collective dependency), then chunk-major ring order so
                # chunk-0 sub-blocks are consumed while chunk 1 gathers.
                visits = [(0, 0), (0, 1)]
                for c in range(2):
                    visits += [(b, c) for b in range(1, NCORES)]
                for v in range(NV // 2):
                    sub = [visits[2 * v], visits[2 * v + 1]]
                    kts, xbs = [], []
                    for si, (b, c) in enumerate(sub):
                        if b == 0:
                            # own K^T already sits in SBUF from phase A
                            kts.append(kst[c])
                        else:
                            kt = ktp.tile([P, DC * CB], F32R, tag="kt",
                                          name="kt")
                            kpid = pid if si == 0 else pid_a
                            (nc.sync if si == 0 else nc.scalar).dma_start(
                                kt.rearrange("p (k c) -> p k c", k=DC),
                                ktb_all[c][
                                    bass_mod.ds(((kpid + b) % NCORES) * D, D), :]
                                .rearrange("(k p) c -> p k c", p=P))
                            kts.append(kt)
                        # PV runs in bf16 (matmul can't mix f32r/bf16); X is
                        # pre-cast to bf16 on the host
                        xb = xbp.tile([P, TC * D], BF16, tag="xb", name="xb")
                        nc.gpsimd.dma_start(
                            xb.rearrange("p (k c) -> p k c", k=TC),
                            xbf_p[bass_mod.ds(
                                ((pid_g + b) % NCORES) * SL + c * CB, CB), :]
                            .rearrange("(k p) c -> p k c", p=P))
                        xbs.append(xb)

                    for m in range(MC):
                        sh_ = [sps.tile([P, CB], F32, tag="s", name="s")
                               for _ in range(2)]
                        mqh = [stp.tile([P, 1], F32, tag=f"mq{s}",
                                        name=f"mq{s}") for s in range(2)]
                        for s in range(2):
                            for k in range(DC):
                                nc.tensor.matmul(
                                    sh_[s][:],
                                    qt[:, k * SL + m * P: k * SL + (m + 1) * P],
                                    kts[s][:, k * CB:(k + 1) * CB],
                                    start=(k == 0), stop=(k == DC - 1),
                                )
                            nc.vector.reduce_max(mqh[s][:], sh_[s][:],
                                                 axis=mybir.AxisListType.X)

                        # online softmax stats; mst ping-pongs on v parity
                        m_old = mst[m][v % 2]
                        mnew = mst[m][(v + 1) % 2]
                        mq = stp.tile([P, 1], F32, tag="mq", name="mq")
                        nc.vector.tensor_max(mq[:], mqh[0][:], mqh[1][:])
                        nc.vector.tensor_max(mnew[:], m_old[:], mq[:])
                        nbias = stp.tile([P, 1], F32, tag="nbias", name="nbias")
                        nc.scalar.mul(nbias[:], mnew[:], -SCALE)
                        # alpha = exp((m_old - mnew)/32)
                        alpha = stp.tile([P, 1], F32, tag="alpha", name="alpha")
                        nc.scalar.activation(alpha[:], m_old[:], ACTF.Exp,
                                             bias=nbias[:], scale=SCALE)

                        # phat = exp(s/32 - mnew/32) in bf16; sums into sq
                        phs = []
                        sqh = [stp.tile([P, 1], F32, tag=f"sq{s}",
                                        name=f"sq{s}") for s in range(2)]
                        for s in range(2):
                            ph = php.tile([P, CB], BF16, tag="ph", name="ph")
                            nc.scalar.activation(ph[:], sh_[s][:], ACTF.Exp,
                                                 bias=nbias[:], scale=SCALE,
                                                 accum_out=sqh[s][:])
                            phs.append(ph)
                        sq = stp.tile([P, 1], F32, tag="sq", name="sq")
                        nc.vector.tensor_add(sq[:], sqh[0][:], sqh[1][:])
                        nc.vector.scalar_tensor_tensor(
                            sig[m][:], sig[m][:], alpha[:], sq[:],
                            op0=ALU.mult, op1=ALU.add)

                        pending.append((phs, alpha, m, v, xbs))
                        if len(pending) > 2:
                            pend_fl = pending.pop(0)
                            flush_dve(pend_fl, flush_pe(pend_fl))
                for pend in pending:
                    flush_dve(pend, flush_pe(pend))

    nc.compile()
    return nc


_PROGRAM = None


def _get_program():
    global _PROGRAM
    if _PROGRAM is None:
        _PROGRAM = build_program()
    return _PROGRAM


def kernel(inputs, rotation_params, entangle_params, _trace=False):
    X = np.ascontiguousarray(np.asarray(inputs, dtype=np.float32))
    R = np.ascontiguousarray(np.asarray(rotation_params, dtype=np.float32))
    E = np.ascontiguousarray(np.asarray(entangle_params, dtype=np.float32))
    assert X.shape == (S, D) and R.shape == (D, D) and E.shape == (D, D)

    import ml_dtypes
    XT = np.ascontiguousarray(X.T)
    Xbf = np.ascontiguousarray(X.astype(ml_dtypes.bfloat16))
    in_maps = []
    for i in range(NCORES):
        in_maps.append({
            "xt": np.ascontiguousarray(XT[:, i * SL:(i + 1) * SL]),
            "r": R,
            "e": E,
            "x": X,
            "xbf": Xbf,
        })

    nc = _get_program()
    res = run_bass_kernel_spmd(nc, in_maps, list(range(NCORES)),
                               trace=_trace)
    out = np.concatenate([res.results[i]["out"] for i in range(NCORES)],
                         axis=0)
    if _trace:
        return out, res
    return out
